# revision 1
# baseline (speedup 1.0000x reference)
"""CenterLoss on 8 Trainium2 NeuronCores.

mean_i ||x_i - centers[labels_i]||^2  with per-sample clip to [1e-12, 1e12].

Sharding (expert/tensor-style class sharding, load-balanced):
  - centers is sharded over classes: core j owns rows [j*12500, (j+1)*12500).
    Each core's device table is [12500 shard | 1 zero row | 128 overflow rows].
  - the batch is routed MoE-style to the core owning each sample's label
    class.  Cores capped at B/8 samples; overflow samples are re-routed to
    under-loaded cores and their (few) center rows are shipped in that
    core's overflow appendix.  With B = 4096 every core computes exactly
    512 samples - no padding waste.
  - each core gathers its 512 centers rows ON DEVICE via indirect DMA
    (one 128-row-offset descriptor-generated transfer per row-tile),
    computes per-sample squared distances, clips, reduces to a partial sum.
  - the 8 partial sums are combined on the host (gather/unshard step).

Per-core device kernel (SPMD, identical program, T row-tiles of 128):
  xa   [128, T*512] f32 : x rows; tile t occupies columns [t*512,(t+1)*512)
                          with sample (t*128+p) in partition p
  idx  [128, T]     i32 : table-local center row per sample (12500 = zero row)
  ctab [12629, 512] f32 : class shard + zero row + overflow appendix
  out  [1, 1]       f32 : sum over samples of clip(||x - c||^2, 1e-12, 1e12)
"""

import os
import sys

import numpy as np

if "/opt/trn_rl_repo" not in sys.path:
    sys.path.insert(0, "/opt/trn_rl_repo")

N_CORES = 8
C = 100000
D = 512
P = 128
CPC = C // N_CORES  # classes per core
OV = 128  # overflow appendix rows
V = CPC + 1 + OV  # device table rows: shard + zero row + appendix
ZERO_ROW = CPC  # index of the all-zero row (pad target)

_compiled = {}
last_results = None  # BassKernelResults of the most recent run (for harnesses)


def _build(T):
    import concourse.bass as bass
    import concourse.tile as tile
    from concourse import bacc, mybir

    nc = bacc.Bacc("TRN2", target_bir_lowering=False, debug=False, num_devices=N_CORES)
    xa_d = nc.dram_tensor("xa", [P, T * D], mybir.dt.float32, kind="ExternalInput").ap()
    idx_d = nc.dram_tensor("idx", [P, T], mybir.dt.int32, kind="ExternalInput").ap()
    ctab_d = nc.dram_tensor("ctab", [V, D], mybir.dt.float32, kind="ExternalInput").ap()
    out_d = nc.dram_tensor("out", [1, 1], mybir.dt.float32, kind="ExternalOutput").ap()

    with tile.TileContext(nc) as tc:
        with (
            tc.tile_pool(name="cpool", bufs=T + 1) as cpool,
            tc.tile_pool(name="dpool", bufs=3) as dpool,
            tc.tile_pool(name="spool", bufs=2) as spool,
            tc.tile_pool(name="small", bufs=1) as small,
            tc.tile_pool(name="psum", bufs=1, space="PSUM") as psum_pool,
        ):
            idx_t = small.tile([P, T], mybir.dt.int32)
            nc.sync.dma_start(idx_t[:], idx_d[:])
            dist = small.tile([P, T], mybir.dt.float32)

            x_all = small.tile([P, T * D], mybir.dt.float32)
            nc.sync.dma_start(x_all[:], xa_d[:])

            for t in range(T):
                sl = slice(t * D, (t + 1) * D)
                # indirect DMA consumes ONE offset per partition: 128 rows/call
                c_t = cpool.tile([P, D], mybir.dt.float32, tag="c")
                nc.gpsimd.indirect_dma_start(
                    out=c_t[:],
                    out_offset=None,
                    in_=ctab_d[:],
                    in_offset=bass.IndirectOffsetOnAxis(ap=idx_t[:, t : t + 1], axis=0),
                )
                diff = dpool.tile([P, D], mybir.dt.float32, tag="diff")
                nc.vector.tensor_tensor(
                    out=diff[:],
                    in0=x_all[:, sl],
                    in1=c_t[:],
                    op=mybir.AluOpType.subtract,
                )
                sq = spool.tile([P, D], mybir.dt.float32, tag="sq")
                nc.scalar.activation(
                    out=sq[:],
                    in_=diff[:],
                    func=mybir.ActivationFunctionType.Square,
                    accum_out=dist[:, t : t + 1],
                )

            # clip+reduce the first T-1 columns while the last tile is still
            # in flight; only the last column's clip+add sits on the tail
            distc = small.tile([P, T - 1], mybir.dt.float32)
            nc.vector.tensor_scalar(
                out=distc[:],
                in0=dist[:, 0 : T - 1],
                scalar1=1e-12,
                scalar2=1e12,
                op0=mybir.AluOpType.max,
                op1=mybir.AluOpType.min,
            )
            s0 = small.tile([P, 1], mybir.dt.float32)
            nc.vector.reduce_sum(out=s0[:], in_=distc[:], axis=mybir.AxisListType.X)
            lastc = small.tile([P, 1], mybir.dt.float32)
            nc.vector.tensor_scalar(
                out=lastc[:],
                in0=dist[:, T - 1 : T],
                scalar1=1e-12,
                scalar2=1e12,
                op0=mybir.AluOpType.max,
                op1=mybir.AluOpType.min,
            )
            s = small.tile([P, 1], mybir.dt.float32)
            nc.vector.tensor_add(out=s[:], in0=s0[:], in1=lastc[:])
            ones = small.tile([P, 1], mybir.dt.float32)
            nc.vector.memset(ones[:], 1.0)
            ps = psum_pool.tile([1, 1], mybir.dt.float32)
            nc.tensor.matmul(ps[:], lhsT=s[:], rhs=ones[:], start=True, stop=True)
            res = small.tile([1, 1], mybir.dt.float32)
            nc.vector.tensor_copy(res[:], ps[:])
            nc.sync.dma_start(out_d[:], res[:])

    nc.compile()
    return nc


def _get_compiled(T):
    if T not in _compiled:
        _compiled[T] = _build(T)
    return _compiled[T]


def _route_balanced(labels, cap):
    """Assign each sample to a core (owner if it has room, else a core with a
    free slot).  Returns per-core sample-index arrays and per-core overflow
    lists (samples whose class lives on another core).  None if the overflow
    appendix would overflow."""
    owner = (labels // CPC).astype(np.int64)
    per_core = []
    overflow = []
    for j in range(N_CORES):
        sel = np.nonzero(owner == j)[0]
        per_core.append(sel[:cap])
        overflow.append(sel[cap:])
    spill = np.concatenate(overflow) if overflow else np.empty(0, np.int64)
    spill_assign = [[] for _ in range(N_CORES)]
    if len(spill):
        free = [cap - len(per_core[j]) for j in range(N_CORES)]
        order = np.argsort(-np.asarray(free))
        pos = 0
        for j in order:
            take = min(free[j], len(spill) - pos)
            if take <= 0:
                continue
            spill_assign[j] = spill[pos : pos + take]
            pos += take
            if max(len(spill_assign[k]) for k in range(N_CORES)) > OV:
                return None
        if pos < len(spill):
            return None
    for j in range(N_CORES):
        if len(spill_assign[j]) > OV:
            return None
    return per_core, spill_assign


def make_in_maps(x, labels, centers):
    """Shard full inputs into per-core input maps. Returns (in_maps, T, B)."""
    x = np.asarray(x, dtype=np.float32)
    labels = np.asarray(labels).astype(np.int64)
    centers = np.asarray(centers, dtype=np.float32)
    B = x.shape[0]

    cap = -(-B // N_CORES)
    cap = -(-cap // P) * P  # per-core sample slots, multiple of 128
    T = cap // P

    routed = _route_balanced(labels, cap)
    if routed is None:
        # degenerate label distribution: fall back to pure route-by-owner
        return _make_in_maps_by_owner(x, labels, centers)
    per_core, spill_assign = routed

    in_maps = []
    for j in range(N_CORES):
        prim = per_core[j]
        spill = np.asarray(spill_assign[j], dtype=np.int64)
        k = len(prim) + len(spill)
        xj = np.zeros((cap, D), np.float32)
        ij = np.full((cap,), ZERO_ROW, np.int32)
        xj[: len(prim)] = x[prim]
        ij[: len(prim)] = (labels[prim] - j * CPC).astype(np.int32)
        ctab = np.zeros((V, D), np.float32)
        ctab[:CPC] = centers[j * CPC : (j + 1) * CPC]
        if len(spill):
            xj[len(prim) : k] = x[spill]
            ij[len(prim) : k] = np.arange(CPC + 1, CPC + 1 + len(spill), dtype=np.int32)
            ctab[CPC + 1 : CPC + 1 + len(spill)] = centers[labels[spill]]
        xa = np.ascontiguousarray(
            xj.reshape(T, P, D).transpose(1, 0, 2).reshape(P, T * D)
        )
        idx = _wrap_idx16(ij, T)
        in_maps.append({"xa": xa, "idx": idx, "ctab": ctab})
    return in_maps, T, B


def _wrap_idx16(ij, T):
    """Index layout for the per-tile indirect gathers: [128, T] int32."""
    return np.ascontiguousarray(ij.astype(np.int32).reshape(T, P).T)


def _make_in_maps_by_owner(x, labels, centers):
    """Fallback: route every sample to its owner core, pad to the max count."""
    B = x.shape[0]
    owner = labels // CPC
    counts = np.bincount(owner, minlength=N_CORES)
    T = max(1, -(-int(counts.max()) // P))
    n_pad = T * P
    in_maps = []
    for j in range(N_CORES):
        sel = np.nonzero(owner == j)[0]
        k = len(sel)
        xj = np.zeros((n_pad, D), np.float32)
        xj[:k] = x[sel]
        ij = np.full((n_pad,), ZERO_ROW, np.int32)
        ij[:k] = (labels[sel] - j * CPC).astype(np.int32)
        ctab = np.zeros((V, D), np.float32)
        ctab[:CPC] = centers[j * CPC : (j + 1) * CPC]
        xa = np.ascontiguousarray(
            xj.reshape(T, P, D).transpose(1, 0, 2).reshape(P, T * D)
        )
        idx = _wrap_idx16(ij, T)
        in_maps.append({"xa": xa, "idx": idx, "ctab": ctab})
    return in_maps, T, B


def kernel(x, labels, centers):
    global last_results
    from concourse.bass_utils import run_bass_kernel_spmd

    in_maps, T, B = make_in_maps(x, labels, centers)
    nc = _get_compiled(T)

    trace = bool(os.environ.get("CENTERLOSS_TRACE"))
    kwargs = {}
    if trace:
        kwargs["tmpdir"] = os.environ.get("CENTERLOSS_TRACE_DIR") or None
    res = run_bass_kernel_spmd(
        nc, in_maps, list(range(N_CORES)), trace=trace, **kwargs
    )
    last_results = res
    total = sum(float(res.results[j]["out"].sum()) for j in range(N_CORES))
    return np.float32(total / B)

